# revision 5
# baseline (speedup 1.0000x reference)
"""Trainium2 Bass kernel for MetaLayer-style GNN message passing (8 cores).

Strategy (edge-parallel by destination node, zero collectives):
  - Host sorts edges by destination (col) and shards both nodes and their
    incoming edges across the 8 cores (core k owns nodes [k*12800,(k+1)*12800)
    and every edge whose col lands there).
  - Host materialises the edge-MLP input feature-major: einT[96, E_slots] =
    [x[row]; x[col]; edge_attr]^T in bf16, padded into fixed-capacity
    64-node groups so the aggregation structure is identical on every core
    (single SPMD program).
  - Device: MM1 (We1, K=96) -> relu+be1 (ACT) -> MM2 (We2 block-diag,
    pair-packed) + be2 (DVE) -> new_edge_attr out; windowed one-hot
    (DVE is_equal vs iota) + PE matmul accumulates aggT per 128-node tile in
    PSUM; node MLP (Wn1/Wn2) consumes [xT;x_lstmT;encT;aggT] per 512 nodes.
"""

import os
import sys

import numpy as np
import ml_dtypes

sys.path.insert(0, "/opt/trn_rl_repo")

import concourse.bass as bass
import concourse.mybir as mybir
from concourse.tile import TileContext, ScopedClock
from concourse.bass_utils import run_bass_kernel_spmd

BF16 = ml_dtypes.bfloat16
bf16 = mybir.dt.bfloat16
f32 = mybir.dt.float32
AF = mybir.ActivationFunctionType
ALU = mybir.AluOpType

N_NODES = 100000
N_EDGES = 1600000
D = 32
H = 64
NC = 8
SHARD_N = 12800          # nodes per core (N padded to 102400)
GROUP_N = 64             # nodes per aggregation group
MAX_WAITS = 1            # walrus in this build: 1 sem-wait per instruction


# --------------------------------------------------------------------------
# workarounds for the walrus sync-wait limit
# --------------------------------------------------------------------------

def _patched_drain_and_barrier(self, tick_clock, wait_clock):
    nc = self.nc
    collector = nc.sync.nop()
    if collector.ins.sync_info is None:
        collector.ins.sync_info = mybir.SyncInfo(on_wait=[], on_update=[])
    wait_clock.add_sem_waits(collector.ins,
                             ScopedClock({None: tick_clock.global_clock}))
    si = collector.ins.sync_info
    waits = list(si.on_wait) if si is not None else []
    if len(waits) > MAX_WAITS:
        si.on_wait = waits[:MAX_WAITS]
        rest = waits[MAX_WAITS:]
        for i in range(0, len(rest), MAX_WAITS):
            nop = nc.sync.nop()
            nop.ins.sync_info = mybir.SyncInfo(
                on_wait=rest[i:i + MAX_WAITS], on_update=[])
    nc.sync.drain()
    nc.all_engine_barrier()
    popped = nc._tile_sem_poison_stack.pop()
    assert popped is self._sem_poison
    nc.clear_and_free_semaphores(list(self.sems.allocated().values()))
    nc.all_engine_barrier()


TileContext._drain_and_barrier = _patched_drain_and_barrier


def _fix_sync_waits(nc, limit=MAX_WAITS):
    """Spill excess per-instruction sem waits onto preceding same-engine NoOps."""
    for fn in nc.m.functions:
        for blk in fn.blocks:
            insts = blk.instructions
            out = []
            changed = False
            for inst in insts:
                si = inst.sync_info
                waits = list(si.on_wait) if si is not None and si.on_wait else []
                if len(waits) > limit:
                    extra = waits[:-limit]
                    for j in range(0, len(extra), limit):
                        nop = mybir.InstNoOp(name=f"I-sw{nc.next_id()}",
                                             ins=[], outs=[], engine=inst.engine)
                        nop.sync_info = mybir.SyncInfo(
                            on_wait=extra[j:j + limit], on_update=[])
                        out.append(nop)
                    si.on_wait = waits[-limit:]
                    changed = True
                out.append(inst)
            if changed:
                blk.instructions = out


# --------------------------------------------------------------------------
# device program
# --------------------------------------------------------------------------

def _build_program(shard_n, J):
    """One SPMD program shared by all cores.

    shard_n: nodes per core (multiple of 512). J: chunks (of 128 edge slots)
    per 64-node group.  E_slots = (shard_n/64)*J*128, a multiple of 1024.
    """
    n_groups = shard_n // GROUP_N
    n_tiles = shard_n // 128
    n_chunks = n_groups * J
    e_slots = n_chunks * 128
    assert e_slots % 1024 == 0
    n_pairs = e_slots // 1024

    nc = bass.Bass()
    dp = nc.declare_dram_parameter
    einT = dp("einT", [96, e_slots], bf16, isOutput=False)
    colloc = dp("colloc", [128, n_chunks], f32, isOutput=False)
    we1 = dp("we1", [96, H], bf16, isOutput=False)
    we2b = dp("we2b", [128, 2 * D], bf16, isOutput=False)
    wn1 = dp("wn1", [128, H], bf16, isOutput=False)
    wn2 = dp("wn2", [H, D], bf16, isOutput=False)
    be1r = dp("be1r", [128, 1], f32, isOutput=False)
    be2r = dp("be2r", [128, 256], bf16, isOutput=False)
    bn1c = dp("bn1c", [H, 1], f32, isOutput=False)
    bn2c = dp("bn2c", [D, 1], f32, isOutput=False)
    iota64 = dp("iota64", [128, 512], bf16, isOutput=False)
    xt3 = dp("xt3", [96, shard_n], bf16, isOutput=False)
    ne_out = dp("ne_out", [n_pairs * 128, 256], bf16, isOutput=True)
    nx_out = dp("nx_out", [D, shard_n], bf16, isOutput=True)

    with TileContext(nc) as tc:
        with tc.tile_pool(name="const", bufs=1) as cpool, \
             tc.tile_pool(name="ein", bufs=3) as einp, \
             tc.tile_pool(name="eh", bufs=3) as ehp, \
             tc.tile_pool(name="ne", bufs=3) as nep, \
             tc.tile_pool(name="oh", bufs=2) as ohp, \
             tc.tile_pool(name="cl", bufs=2) as clp, \
             tc.tile_pool(name="nin", bufs=2) as ninp, \
             tc.tile_pool(name="nh", bufs=2) as nhp, \
             tc.tile_pool(name="nx", bufs=2) as nxp, \
             tc.tile_pool(name="ehps", bufs=2, space="PSUM") as ehps, \
             tc.tile_pool(name="neps", bufs=2, space="PSUM") as neps, \
             tc.tile_pool(name="aggps", bufs=2, space="PSUM") as aggps, \
             tc.tile_pool(name="ndps", bufs=1, space="PSUM") as ndps:

            we1_t = cpool.tile([96, H], bf16)
            nc.sync.dma_start(out=we1_t[:], in_=we1[:])
            we2b_t = cpool.tile([128, 2 * D], bf16)
            nc.sync.dma_start(out=we2b_t[:], in_=we2b[:])
            wn1_t = cpool.tile([128, H], bf16)
            nc.sync.dma_start(out=wn1_t[:], in_=wn1[:])
            wn2_t = cpool.tile([H, D], bf16)
            nc.sync.dma_start(out=wn2_t[:], in_=wn2[:])
            be1_t = cpool.tile([128, 1], f32)
            nc.sync.dma_start(out=be1_t[:], in_=be1r[:])
            be2_t = cpool.tile([128, 256], bf16)
            nc.sync.dma_start(out=be2_t[:], in_=be2r[:])
            bn1_t = cpool.tile([H, 1], f32)
            nc.sync.dma_start(out=bn1_t[:], in_=bn1c[:])
            bn2_t = cpool.tile([D, 1], f32)
            nc.sync.dma_start(out=bn2_t[:], in_=bn2c[:])
            io_t = cpool.tile([128, 512], bf16)
            nc.sync.dma_start(out=io_t[:], in_=iota64[:])

            agg_tiles = {}      # node-tile t -> [128,128] psum tile (rows 96:)
            nin_tiles = {}      # nt4 -> [128, 512] sbuf staging

            def node_mlp(nt4):
                """Run node MLP for node tiles 4*nt4..4*nt4+3."""
                nin_t = nin_tiles.pop(nt4)
                nps = ndps.tile([H, 512], f32, tag="nh")
                nc.tensor.matmul(out=nps[:], lhsT=wn1_t[:], rhs=nin_t[:],
                                 start=True, stop=True)
                nh_t = nhp.tile([H, 512], bf16)
                nc.scalar.activation(nh_t[:], nps[:], AF.Relu,
                                     bias=bn1_t[:, 0:1], scale=1.0)
                xps = ndps.tile([D, 512], f32, tag="nx")
                nc.tensor.matmul(out=xps[:], lhsT=wn2_t[:], rhs=nh_t[:],
                                 start=True, stop=True)
                nx_t = nxp.tile([D, 512], bf16)
                nc.scalar.activation(nx_t[:], xps[:], AF.Identity,
                                     bias=bn2_t[:, 0:1], scale=1.0)
                nc.sync.dma_start(out=nx_out[:, nt4 * 512:(nt4 + 1) * 512],
                                  in_=nx_t[:])

            for b in range(n_pairs):
                # ---- load einT pair [96, 1024] ----
                ein_t = einp.tile([96, 1024], bf16)
                nc.sync.dma_start(out=ein_t[:],
                                  in_=einT[:, b * 1024:(b + 1) * 1024])
                # ---- colloc batch (8 chunks per pair; load 64 per 8 pairs) ----
                if b % 8 == 0:
                    cl_t = clp.tile([128, 64], f32)
                    hi = min((b + 8) * 8, n_chunks)
                    nc.sync.dma_start(out=cl_t[:, 0:hi - b * 8],
                                      in_=colloc[:, b * 8:hi])
                # ---- MM1: ehT [128, 512] (A rows 0:64, B rows 64:128) ----
                eh_ps = ehps.tile([128, 512], f32, tag="eh")
                nc.tensor.matmul(out=eh_ps[0:64, :], lhsT=we1_t[:],
                                 rhs=ein_t[:, 0:512], start=True, stop=True,
                                 tile_position=(0, 0))
                nc.tensor.matmul(out=eh_ps[64:128, :], lhsT=we1_t[:],
                                 rhs=ein_t[:, 512:1024], start=True, stop=True,
                                 tile_position=(0, 64))
                eh_t = ehp.tile([128, 512], bf16)
                nc.scalar.activation(eh_t[:], eh_ps[:], AF.Relu,
                                     bias=be1_t[:, 0:1], scale=1.0)
                # ---- MM2 (pair-packed block-diag We2) ----
                ne_ps = neps.tile([128, 256], f32, tag="ne")
                for c in range(4):
                    nc.tensor.matmul(out=ne_ps[:, 64 * c:64 * (c + 1)],
                                     lhsT=eh_t[:, 128 * c:128 * (c + 1)],
                                     rhs=we2b_t[:], start=True, stop=True)
                ne_t = nep.tile([128, 256], bf16)
                nc.vector.tensor_tensor(out=ne_t[:], in0=ne_ps[:],
                                        in1=be2_t[:], op=ALU.add)
                nc.sync.dma_start(out=ne_out[b * 128:(b + 1) * 128, :],
                                  in_=ne_t[:])
                # ---- onehot for the pair's 8 chunks ----
                oh_t = ohp.tile([128, 512], bf16)
                nc.vector.tensor_tensor(
                    out=oh_t[:],
                    in0=cl_t[:, (b % 8) * 8:(b % 8) * 8 + 8].to_broadcast(
                        [128, 8, GROUP_N]),
                    in1=io_t[:], op=ALU.is_equal)
                # ---- agg matmuls (chunk q = 8b + cp) ----
                for cp in range(8):
                    q = 8 * b + cp
                    g = q // J
                    t = g // 2
                    col_off = GROUP_N * (g % 2)
                    if t not in agg_tiles:
                        agg_tiles[t] = aggps.tile([128, 128], f32,
                                                  name=f"agg{t}", tag="agg")
                    if cp < 4:
                        ne_sl = ne_t[:, 64 * cp:64 * cp + 32]
                    else:
                        ne_sl = ne_t[:, 64 * (cp - 4) + 32:64 * (cp - 4) + 64]
                    nc.tensor.matmul(
                        out=agg_tiles[t][96:128, col_off:col_off + GROUP_N],
                        lhsT=ne_sl, rhs=oh_t[:, 64 * cp:64 * (cp + 1)],
                        start=(q % J == 0), stop=(q % J == J - 1),
                        tile_position=(0, 96))
                    # group done & odd -> node tile t complete
                    if q % J == J - 1 and g % 2 == 1:
                        nt4 = t // 4
                        if nt4 not in nin_tiles:
                            nin_t = ninp.tile([128, 512], bf16, tag="nin")
                            nin_tiles[nt4] = nin_t
                            nc.sync.dma_start(
                                out=nin_t[0:96, :],
                                in_=xt3[:, nt4 * 512:(nt4 + 1) * 512])
                        agg_t = agg_tiles.pop(t)
                        nc.scalar.activation(
                            nin_tiles[nt4][96:128, 128 * (t % 4):128 * (t % 4 + 1)],
                            agg_t[96:128, :], AF.Copy, bias=0.0, scale=1.0)
                        if t % 4 == 3:
                            node_mlp(nt4)

    _fix_sync_waits(nc)
    return nc


# --------------------------------------------------------------------------
# host side
# --------------------------------------------------------------------------

def _prep(x, x_lstm, enc, edge_index, edge_attr, We1, be1, We2, be2,
          Wn1, bn1, Wn2, bn2, shard_n=None):
    if shard_n is None:
        shard_n = SHARD_N
    """Shard + build per-core input maps. Returns (J, in_maps, meta)."""
    n_cores = NC
    n_pad = shard_n * n_cores
    row = np.asarray(edge_index[0], dtype=np.int64)
    col = np.asarray(edge_index[1], dtype=np.int64)
    E = row.shape[0]

    n_groups = shard_n // GROUP_N
    total_groups = n_cores * n_groups
    gid = col // GROUP_N                      # global group of each edge
    counts = np.bincount(gid, minlength=total_groups)
    J = int(np.ceil(counts.max() / 128.0))
    J = max(J, 1)
    cap = J * 128
    n_chunks = n_groups * J
    e_slots = n_chunks * 128
    n_pairs = e_slots // 1024

    # slot assignment: edges sorted by group; each group g occupies
    # slots [g_local*cap, g_local*cap+count) on its core.
    order = np.argsort(gid, kind="stable")
    g_sorted = gid[order]
    # position within group
    grp_starts = np.zeros(total_groups + 1, dtype=np.int64)
    np.cumsum(counts, out=grp_starts[1:])
    pos_in_grp = np.arange(E, dtype=np.int64) - grp_starts[g_sorted]
    core_of = g_sorted // n_groups
    glocal = g_sorted % n_groups
    slot = glocal * cap + pos_in_grp          # slot within the core
    # per sorted edge: core_of, slot
    x = np.asarray(x, dtype=np.float32)
    x_lstm = np.asarray(x_lstm, dtype=np.float32)
    enc = np.asarray(enc, dtype=np.float32)
    edge_attr = np.asarray(edge_attr, dtype=np.float32)
    row_s = row[order]
    col_s = col[order]
    ea_s = edge_attr[order]

    xb = x.astype(BF16)
    x3 = np.concatenate([x, x_lstm, enc], axis=1).astype(BF16)  # [N, 96]

    # weight/bias constants (shared across cores)
    we1_h = np.asarray(We1, np.float32).astype(BF16)            # [96, 64]
    we2 = np.asarray(We2, np.float32).astype(BF16)              # [64, 32]
    we2b = np.zeros((128, 2 * D), BF16)
    we2b[0:64, 0:32] = we2
    we2b[64:128, 32:64] = we2
    wn1_h = np.asarray(Wn1, np.float32).astype(BF16)            # [128, 64]
    wn2_h = np.asarray(Wn2, np.float32).astype(BF16)            # [64, 32]
    be1r = np.tile(np.asarray(be1, np.float32), 2)[:, None]     # [128, 1]
    be2r = np.tile(np.asarray(be2, np.float32).astype(BF16), 8)[None, :].repeat(128, 0)
    bn1c = np.asarray(bn1, np.float32)[:, None]
    bn2c = np.asarray(bn2, np.float32)[:, None]
    iota = np.tile(np.arange(GROUP_N, dtype=np.float32).astype(BF16),
                   8)[None, :].repeat(128, 0)                   # [128, 512]

    in_maps = []
    meta_slots = []
    for k in range(n_cores):
        m = core_of == k
        sl = slot[m]
        einT = np.zeros((96, e_slots), BF16)
        einT[0:32, sl] = xb[row_s[m]].T
        einT[32:64, sl] = xb[col_s[m]].T
        einT[64:96, sl] = ea_s[m].astype(BF16).T
        colloc = np.full((128, n_chunks), -1000.0, np.float32)
        colloc_flat = colloc.reshape(128, n_chunks)
        # slot s -> (chunk s//128, partition s%128)
        colloc_flat[sl % 128, sl // 128] = (col_s[m] % GROUP_N).astype(np.float32)
        # node features transposed for this shard (pad beyond N with zeros)
        xt3 = np.zeros((96, shard_n), BF16)
        lo, hi = k * shard_n, min((k + 1) * shard_n, N_NODES)
        if hi > lo:
            xt3[:, 0:hi - lo] = x3[lo:hi].T
        in_maps.append(dict(
            einT=einT, colloc=colloc, we1=we1_h, we2b=we2b, wn1=wn1_h,
            wn2=wn2_h, be1r=be1r.astype(np.float32), be2r=be2r,
            bn1c=bn1c, bn2c=bn2c, iota64=iota, xt3=xt3))
        meta_slots.append((m, sl))
    meta = dict(order=order, meta_slots=meta_slots, e_slots=e_slots,
                n_pairs=n_pairs, shard_n=shard_n, E=E)
    return J, in_maps, meta


def _decode(results, meta):
    """Assemble full new_x [N,32] f32 and new_edge_attr [E,32] f32."""
    E = meta["E"]
    n_pairs = meta["n_pairs"]
    shard_n = meta["shard_n"]
    new_x = np.empty((N_NODES, D), np.float32)
    new_edge = np.empty((E, D), np.float32)
    order = meta["order"]
    for k, r in enumerate(results):
        lo, hi = k * shard_n, min((k + 1) * shard_n, N_NODES)
        if hi > lo:
            new_x[lo:hi] = r["nx_out"].astype(np.float32).T[0:hi - lo]
        # ne_out [n_pairs*128, 256] -> slots
        ne = r["ne_out"].reshape(n_pairs, 128, 4, 2, D)
        # slot s = 1024b + 512*half + 128c + p  <- ne[b, p, c, half, :]
        ne = ne.transpose(0, 3, 2, 1, 4).reshape(n_pairs * 1024, D)
        m, sl = meta["meta_slots"][k]
        new_edge[order[m]] = ne[sl].astype(np.float32)
    return new_x, new_edge


_PROGRAM_CACHE = {}


def kernel(x, x_lstm, encoded_z_gnss, edge_index, edge_attr,
           node_indexes_related_to_agent=None,
           edge_indexes_related_to_agent=None,
           We1=None, be1=None, We2=None, be2=None,
           Wn1=None, bn1=None, Wn2=None, bn2=None):
    J, in_maps, meta = _prep(x, x_lstm, encoded_z_gnss, edge_index, edge_attr,
                             We1, be1, We2, be2, Wn1, bn1, Wn2, bn2)
    key = (SHARD_N, J)
    if key not in _PROGRAM_CACHE:
        _PROGRAM_CACHE[key] = _build_program(SHARD_N, J)
    nc = _PROGRAM_CACHE[key]
    res = run_bass_kernel_spmd(nc, in_maps, list(range(NC)))
    new_x, new_edge = _decode(res.results, meta)
    return new_x, new_edge
